# revision 25
# baseline (speedup 1.0000x reference)
"""Channel-self-attention (LayerNorm + grouped-1x1-qkv + channel softmax attn
+ residual) on 8 TRN2 NeuronCores.

Strategy (v3): pair-sharding — 2 cores per batch, each core owns one
spatial half (16384 of 32768). One ~34 KB 2-rank Mesh AllReduce per core.

Per core:
 - x half-shard [256, 16384] bf16 resident in SBUF (channel-major)
 - host also sends x TRANSPOSED (spatial-major, bf16) packed per 128-row
   stile as [x_A(86) | g2 gb g (3) | x_K(86)] so the Gram matmul needs NO
   on-chip transposes:
     lhsT = [g2 gb g | g2*x_K]   (g2*x_K built by 8 bulk chunk DVE mults
                                  against a host-replicated gamma^2 tile)
     rhs  = the raw packed stile
     out  = [89,175]: rows 0..2 x cols 0..85 = tgh_A, rows 3..88 = P^T,
            rows 0..2 x cols 89..174 = tgh_K
 - stats: Sum x via DVE reduce, Sum x^2 via Scalar Square+accum (idle
   engine), replacing bn_stats
 - ONE AllReduce (Gram + tgh + stats, 33.7 KB) within the batch pair
 - logits from the Gram expansion of the LayerNorm algebra; softmax
   normalization folded into att before the transpose, so the epilogue is
   a plain  out = x + att_nrm @ v  residual add (split DVE/GpSimd), with
   bf16 output upcast on host
"""
import sys

sys.path.insert(0, "/opt/trn_rl_repo")

import numpy as np
import ml_dtypes

B, C = 4, 256
S = 32 * 32 * 32          # 32768 global spatial
NCORES = 8
SH = S // 2               # 16384 per-core spatial half
NST = SH // 128           # 128 stiles
NCH = 8                   # Gram stream chunks
CST = NST // NCH          # 16 stiles per chunk
EPS = 1e-5
SCALE = float(S) ** -0.5

_BF = ml_dtypes.bfloat16

_cache = {}


def _build_program():
    from contextlib import ExitStack
    import concourse.bass as bass
    import concourse.bacc as bacc
    import concourse.tile as tile
    from concourse import mybir, masks

    f32 = mybir.dt.float32
    bf16 = mybir.dt.bfloat16
    AF = mybir.ActivationFunctionType
    OP = mybir.AluOpType
    AX = mybir.AxisListType

    nc = bacc.Bacc(
        "TRN2",
        target_bir_lowering=False,
        debug=False,
        enable_asserts=False,
        num_devices=NCORES,
    )

    # ---------------- DRAM I/O ----------------
    xs_d = nc.dram_tensor("xs", [C, SH], bf16, kind="ExternalInput")
    xt_d = nc.dram_tensor("xt", [128, NST * 176], bf16, kind="ExternalInput")
    g2e_d = nc.dram_tensor("g2e", [128, NST * 90], bf16, kind="ExternalInput")
    gb1r_d = nc.dram_tensor("gb1r", [3, SH], bf16, kind="ExternalInput")
    eqt_d = nc.dram_tensor("eqt", [97, C], f32, kind="ExternalInput")
    ekt_d = nc.dram_tensor("ekt", [86, C], f32, kind="ExternalInput")
    w0_d = nc.dram_tensor("w0", [128, 2 * 87], bf16, kind="ExternalInput")
    bk_d = nc.dram_tensor("bk", [1, C], f32, kind="ExternalInput")
    sc_d = nc.dram_tensor("sc", [1, 8], f32, kind="ExternalInput")
    out_d = nc.dram_tensor("out", [C, SH], bf16, kind="ExternalOutput")

    # AR payload layout (f32 words). M = [90,175] Gram PSUM; lhsT col 89
    # is ones so M row 89 = per-channel Sum x for A (cols 0:86) / K (89:175).
    #   [0 : 7740)        M[0:90, 0:86] row-major (tgh_A 0:3, P^T 3:89, SxA 89)
    #   [7740 : 7998)     M[0:3, 89:175] row-major (tgh_K)
    #   [7998 : 8084)     M[89, 89:175]  (Sum x_K)
    #   [8084 : 8170)     Sum x_V (DVE reduce over partitions 42:128 of ct1)
    #   [8170 : 8426)     Sum x^2 per channel
    PTOT = 8426

    with tile.TileContext(nc) as tc, ExitStack() as ctx:
        const = ctx.enter_context(tc.tile_pool(name="const", bufs=1))
        xpool = ctx.enter_context(tc.tile_pool(name="xpool", bufs=1))
        xtp = ctx.enter_context(tc.tile_pool(name="xtp", bufs=2))
        utp = ctx.enter_context(tc.tile_pool(name="utp", bufs=2))
        g2p = ctx.enter_context(tc.tile_pool(name="g2p", bufs=1))
        rhsp = ctx.enter_context(tc.tile_pool(name="rhsp", bufs=1))
        osml = ctx.enter_context(tc.tile_pool(name="osml", bufs=2))
        small = ctx.enter_context(tc.tile_pool(name="small", bufs=2))
        dram = ctx.enter_context(tc.tile_pool(name="dram", bufs=1, space="DRAM"))

        # ------------- constants / inputs to SBUF -------------
        ident = const.tile([128, 128], f32)
        masks.make_identity(nc, ident[:])
        ident_bf = const.tile([128, 128], bf16)
        masks.make_identity(nc, ident_bf[:])
        def dram_bcast(dst, src_d, nparts, nfree, off=0):
            nc.gpsimd.dma_start(
                out=dst,
                in_=bass.AP(tensor=src_d, offset=off,
                            ap=[[0, nparts], [1, nfree]]))

        # Gram streams: xt chunks on gpsimd queue; g2e resident via scalar
        g2e_sb = g2p.tile([128, NST, 90], bf16)
        nc.scalar.dma_start(out=g2e_sb[:], in_=g2e_d.ap())
        xt_sb, u2t_sb = [], []
        for c in range(NCH):
            t = xtp.tile([128, CST, 176], bf16, tag="xt", name=f"xt{c}")
            nc.gpsimd.dma_start(
                out=t[:],
                in_=xt_d[:, c * CST * 176:(c + 1) * CST * 176])
            xt_sb.append(t)
            u = utp.tile([128, CST, 90], bf16, tag="u2t", name=f"u2{c}")
            u2t_sb.append(u)

        # x resident bf16 [128, 2, 16384], 8 chunked DMAs for early stats
        x_sb = xpool.tile([128, 2, SH], bf16)
        for ct in range(2):
            for cc in range(4):
                nc.sync.dma_start(
                    out=x_sb[:, ct, cc * 4096:(cc + 1) * 4096],
                    in_=xs_d[ct * 128:(ct + 1) * 128,
                             cc * 4096:(cc + 1) * 4096])

        # ------------- Gram over 128 stiles (8 chunks) -------------
        bncP_in = dram.tile([PTOT], f32)
        bncP_out = dram.tile([PTOT], f32)

        with tc.tile_pool(name="s1ps", bufs=1, space="PSUM") as stg1ps:
            ptk_ps = stg1ps.tile([90, 175], f32)
            with tc.high_priority():
                for c in range(NCH):
                    nc.vector.tensor_tensor(
                        out=u2t_sb[c][:], in0=xt_sb[c][:, :, 86:176],
                        in1=g2e_sb[:, c * CST:(c + 1) * CST, :], op=OP.mult)
                    for j in range(CST):
                        st = c * CST + j
                        nc.tensor.matmul(
                            ptk_ps[:], lhsT=u2t_sb[c][:, j, :],
                            rhs=xt_sb[c][:, j, 0:175],
                            start=(st == 0), stop=(st == NST - 1))

            # ------- stats: Sum x_V (DVE), Sum x^2 (Scalar) -------
            sumsV = const.tile([128, 1], f32)
            sVp = const.tile([128, 4], f32)
            sqp = const.tile([128, 8], f32)
            sqs_sb = const.tile([128, 2], f32)
            for cc in range(4):
                nc.vector.reduce_sum(
                    sVp[:, cc:cc + 1],
                    x_sb[:, 1, cc * 4096:(cc + 1) * 4096], axis=AX.X)
            nc.vector.reduce_sum(sumsV[:], sVp[:], axis=AX.X)
            for ct in range(2):
                for cc in range(4):
                    scr = osml.tile([128, 4096], bf16, tag="sqscr", bufs=1,
                                    name=f"sq{ct}{cc}")
                    nc.scalar.activation(
                        out=scr[:], in_=x_sb[:, ct, cc * 4096:(cc + 1) * 4096],
                        func=AF.Square,
                        accum_out=sqp[:, 4 * ct + cc:4 * ct + cc + 1])
                nc.vector.reduce_sum(
                    sqs_sb[:, ct:ct + 1], sqp[:, 4 * ct:4 * ct + 4], axis=AX.X)

            ptk_sb = small.tile([90, 86], f32, tag="ptksb", bufs=1)
            nc.scalar.copy(ptk_sb[:], ptk_ps[0:90, 0:86])
            ptk3_sb = small.tile([90, 86], f32, tag="ptk3sb", bufs=1)
            nc.scalar.copy(ptk3_sb[:], ptk_ps[0:90, 89:175])

        # ------------- AllReduce within the batch pair -------------
        nc.gpsimd.dma_start(
            out=bncP_in[0:7740].rearrange("(p f) -> p f", f=86),
            in_=ptk_sb[:])
        nc.gpsimd.dma_start(
            out=bncP_in[7740:7998].rearrange("(p f) -> p f", f=86),
            in_=ptk3_sb[0:3, :])
        nc.gpsimd.dma_start(
            out=bncP_in[7998:8084].rearrange("(p f) -> p f", f=86),
            in_=ptk3_sb[89:90, :])
        nc.gpsimd.dma_start(
            out=bncP_in[8084:8170].rearrange("(p f) -> p f", f=1),
            in_=sumsV[42:128, :])
        nc.gpsimd.dma_start(
            out=bncP_in[8170:8426].rearrange("(t p) -> p t", p=128),
            in_=sqs_sb[:])
        nc.gpsimd.collective_compute(
            "AllReduce", OP.add,
            replica_groups=[[0, 1], [2, 3], [4, 5], [6, 7]],
            ins=[bncP_in[:].opt()], outs=[bncP_out[:].opt()])

        # ------------- late consts (DMA slots free after the Gram) ---------
        eqt_sb = const.tile([97, C], f32)
        nc.sync.dma_start(out=eqt_sb[:], in_=eqt_d.ap())
        ekt_sb = const.tile([86, C], f32)
        nc.sync.dma_start(out=ekt_sb[:], in_=ekt_d.ap())
        w0_sb = const.tile([128, 2, 87], bf16)
        nc.sync.dma_start(out=w0_sb[:], in_=w0_d.ap())
        bk_bc = const.tile([128, C], f32)
        dram_bcast(bk_bc[:], bk_d, 128, C)
        sc_bc = const.tile([128, 8], f32)
        dram_bcast(sc_bc[:], sc_d, 128, 8)
        gam_bc = const.tile([86, SH], bf16)
        dram_bcast(gam_bc[:], gb1r_d, 86, SH, off=SH)
        nc.vector.tensor_scalar_mul(gam_bc[:], gam_bc[:], -1.0)

        # ------------- rhs for M2 -------------
        # rows 0..85 = gamma*x_V (ch 170..255), 86..88 = [ones, -gamma, beta]
        rhs_m2 = rhsp.tile([128, SH], bf16)
        nc.gpsimd.dma_start(out=rhs_m2[0:86, :], in_=x_sb[42:128, 1, :])
        nc.vector.tensor_tensor(
            out=rhs_m2[0:86, :], in0=rhs_m2[0:86, :], in1=gam_bc[0:86, :],
            op=OP.mult)
        nc.gpsimd.dma_start(out=rhs_m2[86:89, :], in_=gb1r_d.ap())

        # ------------- DMA back -------------
        pt_back = const.tile([86, 86], f32)
        nc.sync.dma_start(
            out=pt_back[:],
            in_=bass.AP(tensor=bncP_out.tensor,
                        offset=bncP_out.offset + 3 * 86,
                        ap=[[86, 86], [1, 86]]))  # P^T rows 3..88 of block1
        tga = const.tile([86, 3], f32)
        nc.sync.dma_start(
            out=tga[:],
            in_=bass.AP(tensor=bncP_out.tensor, offset=bncP_out.offset,
                        ap=[[1, 86], [86, 3]]))
        tgk = const.tile([86, 3], f32)
        nc.sync.dma_start(
            out=tgk[:],
            in_=bass.AP(tensor=bncP_out.tensor,
                        offset=bncP_out.offset + 7740,
                        ap=[[1, 86], [86, 3]]))
        # stats cols: [p, {Sx,Sxx} x {A,K,V}]
        sAK = const.tile([86, 6], f32)
        nc.sync.dma_start(
            out=sAK[:, 0:1],
            in_=bass.AP(tensor=bncP_out.tensor,
                        offset=bncP_out.offset + 89,
                        ap=[[90, 86], [1, 1]]))       # Sx_A = block1 col 89
        nc.sync.dma_start(
            out=sAK[:, 1:3],
            in_=bass.AP(tensor=bncP_out.tensor,
                        offset=bncP_out.offset + 7998,
                        ap=[[1, 86], [86, 2]]))       # Sx_K, Sx_V
        nc.sync.dma_start(
            out=sAK[:, 3:6],
            in_=bass.AP(tensor=bncP_out.tensor,
                        offset=bncP_out.offset + 8170,
                        ap=[[1, 86], [85, 3]]))       # Sxx A/K/V

        invS = 1.0 / float(S)

        # --- per-channel LayerNorm scalars ---
        mAK = small.tile([86, 3], f32, tag="mAK")
        nc.vector.tensor_scalar(
            out=mAK[:], in0=sAK[:, 0:3], scalar1=invS, scalar2=None,
            op0=OP.mult)
        vAK = small.tile([86, 3], f32, tag="vAK")
        nc.vector.tensor_scalar(
            out=vAK[:], in0=sAK[:, 3:6], scalar1=invS, scalar2=EPS,
            op0=OP.mult, op1=OP.add)
        msq = small.tile([86, 3], f32, tag="msq")
        nc.vector.tensor_mul(msq[:], mAK[:], mAK[:])
        nc.vector.tensor_sub(vAK[:], vAK[:], msq[:])
        nc.scalar.activation(out=vAK[:], in_=vAK[:], func=AF.Sqrt)
        rAK = small.tile([86, 3], f32, tag="rAK")
        nc.vector.reciprocal(rAK[:], vAK[:])
        invrV = small.tile([86, 1], f32, tag="invrV")
        nc.vector.reciprocal(invrV[:], rAK[:, 2:3])
        mvinv_bf = small.tile([86, 2], bf16, tag="mvinv")
        nc.vector.tensor_copy(mvinv_bf[:, 0:1], mAK[:, 2:3])
        nc.vector.tensor_copy(mvinv_bf[:, 1:2], invrV[:])
        rv_ext = small.tile([128, 1], f32, tag="rvext")
        nc.vector.memset(rv_ext[64:128, :], 1.0)
        nc.vector.tensor_copy(rv_ext[0:86, :], rAK[:, 2:3])

        tA, gA, hA = tga[:, 0:1], tga[:, 1:2], tga[:, 2:3]
        tK, gK, hK = tgk[:, 0:1], tgk[:, 1:2], tgk[:, 2:3]
        mA, mK = mAK[:, 0:1], mAK[:, 1:2]
        rA, rK = rAK[:, 0:1], rAK[:, 1:2]
        scG1 = sc_bc[0:86, 0:1]
        scG2 = sc_bc[0:86, 1:2]
        scGb = sc_bc[0:86, 2:3]
        scB1 = sc_bc[0:86, 3:4]
        scBb = sc_bc[0:86, 4:5]

        ntK = small.tile([86, 1], f32, tag="ntK")
        nc.vector.tensor_scalar_mul(ntK[:], tK, -1.0)
        nmK = small.tile([86, 1], f32, tag="nmK")
        nc.vector.tensor_scalar_mul(nmK[:], mK, -1.0)
        g2mK = small.tile([86, 1], f32, tag="g2mK")
        nc.vector.tensor_scalar(
            out=g2mK[:], in0=mK, scalar1=scG2, scalar2=None, op0=OP.mult)
        t3c = small.tile([86, 1], f32, tag="t3c")
        nc.vector.tensor_scalar(
            out=t3c[:], in0=mK, scalar1=scGb, scalar2=None, op0=OP.mult)
        nc.vector.tensor_sub(t3c[:], gK, t3c[:])
        nc.vector.tensor_mul(t3c[:], rK, t3c[:])
        t2c = small.tile([86, 1], f32, tag="t2c")
        nc.vector.tensor_scalar(
            out=t2c[:], in0=mA, scalar1=scGb, scalar2=None, op0=OP.mult)
        nc.vector.tensor_sub(t2c[:], gA, t2c[:])
        nc.vector.tensor_mul(t2c[:], rA, t2c[:])
        syA = small.tile([86, 1], f32, tag="syA")
        nc.vector.tensor_scalar(
            out=syA[:], in0=mA, scalar1=scG1, scalar2=None, op0=OP.mult)
        nc.vector.tensor_sub(syA[:], hA, syA[:])
        nc.vector.tensor_mul(syA[:], rA, syA[:])
        nc.vector.tensor_scalar(
            out=syA[:], in0=syA[:], scalar1=scB1, scalar2=None, op0=OP.add)
        syK = small.tile([86, 1], f32, tag="syK")
        nc.vector.tensor_scalar(
            out=syK[:], in0=mK, scalar1=scG1, scalar2=None, op0=OP.mult)
        nc.vector.tensor_sub(syK[:], hK, syK[:])
        nc.vector.tensor_mul(syK[:], rK, syK[:])
        nc.vector.tensor_scalar(
            out=syK[:], in0=syK[:], scalar1=scB1, scalar2=None, op0=OP.add)

        with tc.tile_pool(name="psG1", bufs=1, space="PSUM") as psG1, \
             tc.tile_pool(name="psG2", bufs=1, space="PSUM") as psG2, \
             tc.tile_pool(name="psLog", bufs=2, space="PSUM") as psLog:

            # rows (mA, tA, rA, term2) -> transpose -> DRAM -> one bcast DMA
            pack = small.tile([86, 4], f32, tag="pack")
            nc.vector.tensor_copy(pack[:, 0:1], mA)
            nc.vector.tensor_copy(pack[:, 1:2], tA)
            nc.vector.tensor_copy(pack[:, 2:3], rA)
            nc.vector.tensor_copy(pack[:, 3:4], t2c[:])
            packT_ps = psG1.tile([4, 86], f32, tag="pT")
            nc.tensor.transpose(packT_ps[:], pack[:], ident[0:86, 0:86])
            packT = small.tile([4, 86], f32, tag="packT")
            nc.scalar.copy(packT[:], packT_ps[:])
            rows_d = dram.tile([4, 86], f32, tag="rowsd")
            nc.gpsimd.dma_start(out=rows_d[:], in_=packT[:])
            bc4 = small.tile([86, 4, 86], f32, tag="bc4")
            nc.gpsimd.dma_start(
                out=bc4[:],
                in_=bass.AP(tensor=rows_d.tensor, offset=rows_d.offset,
                            ap=[[0, 86], [86, 4], [1, 86]]))

            # --- syy ---
            syy = small.tile([86, 97], f32, tag="syy")
            nc.vector.memset(syy[:, 86:96], 0.0)
            nc.vector.scalar_tensor_tensor(
                out=syy[:, 0:86], in0=bc4[:, 0, :], scalar=ntK[:],
                in1=pt_back[:], op0=OP.mult, op1=OP.add)
            nc.vector.scalar_tensor_tensor(
                out=syy[:, 0:86], in0=bc4[:, 1, :], scalar=nmK[:],
                in1=syy[:, 0:86], op0=OP.mult, op1=OP.add)
            nc.vector.scalar_tensor_tensor(
                out=syy[:, 0:86], in0=bc4[:, 0, :], scalar=g2mK[:],
                in1=syy[:, 0:86], op0=OP.mult, op1=OP.add)
            nc.vector.scalar_tensor_tensor(
                out=syy[:, 0:86], in0=bc4[:, 2, :], scalar=rK,
                in1=syy[:, 0:86], op0=OP.mult, op1=OP.mult)
            nc.vector.tensor_add(syy[:, 0:86], syy[:, 0:86], bc4[:, 3, :])
            nc.vector.tensor_scalar(
                out=syy[:, 0:86], in0=syy[:, 0:86], scalar1=t3c[:],
                scalar2=scBb, op0=OP.add, op1=OP.add)
            nc.vector.tensor_copy(syy[:, 96:97], syK[:])

            # --- logits + softmax (recip folded into att) ---
            u_ps = psG2.tile([97, C], f32, tag="uP")
            nc.tensor.matmul(u_ps[:], lhsT=syy[:], rhs=ekt_sb[:],
                             start=True, stop=True)
            u_ext = small.tile([128, C], f32, tag="uext")
            nc.vector.memset(u_ext[64:128, :], 0.0)
            nc.vector.scalar_tensor_tensor(
                out=u_ext[0:86, :], in0=bk_bc[0:86, :], scalar=syA[:],
                in1=u_ps[0:86, :], op0=OP.mult, op1=OP.add)
            nc.vector.tensor_scalar_mul(
                u_ext[96:97, :], bk_bc[96:97, :], float(S))
            nc.vector.tensor_add(u_ext[96:97, :], u_ext[96:97, :],
                                 u_ps[96:97, :])

            att_nrm = []
            recip2 = small.tile([128, 2], f32, tag="recip2")
            z2 = small.tile([128, 2], f32, tag="z2")
            for it in range(2):
                log_ps = psLog.tile([128, 512], f32, tag="lg", name=f"lg{it}")
                nc.tensor.matmul(
                    log_ps[:, 0:C], lhsT=eqt_sb[:, it * 128:(it + 1) * 128],
                    rhs=u_ext[0:97, :], start=True, stop=True)
                rmax = small.tile([128, 1], f32, tag="rmax", name=f"rm{it}")
                nc.vector.reduce_max(rmax[:], log_ps[:, 0:C], axis=AX.X)
                nbias = small.tile([128, 1], f32, tag="nbias", name=f"nb{it}")
                nc.vector.tensor_scalar_mul(nbias[:], rmax[:], -SCALE)
                a_sb = small.tile([128, C], bf16, tag=f"attsb{it}",
                                  name=f"att{it}")
                nc.scalar.activation(
                    out=a_sb[:], in_=log_ps[:, 0:C], func=AF.Exp,
                    bias=nbias[:], scale=SCALE, accum_out=z2[:, it:it + 1])
                nc.vector.reciprocal(recip2[:, it:it + 1], z2[:, it:it + 1])
                a_nr = small.tile([128, C], bf16, tag=f"anrm{it}",
                                  name=f"an{it}")
                nc.scalar.activation(
                    out=a_nr[:], in_=a_sb[:], func=AF.Copy,
                    scale=recip2[:, it:it + 1])
                att_nrm.append(a_nr)

        # --- NT: lhs_m2 [89 rows, 256 q-ch] ---
        psNtc = ctx.enter_context(tc.tile_pool(name="psNtc", bufs=1,
                                               space="PSUM"))
        psAt = ctx.enter_context(tc.tile_pool(name="psAt", bufs=2,
                                              space="PSUM"))
        psO = ctx.enter_context(tc.tile_pool(name="psO", bufs=2,
                                             space="PSUM"))

        ntc_ps = psNtc.tile([128, C], f32, tag="ntc")
        for jt in range(2):
            at_ps = psAt.tile([128, C], bf16, tag="atp", name=f"atp{jt}")
            for it in range(2):
                nc.tensor.transpose(
                    at_ps[:, it * 128:(it + 1) * 128],
                    att_nrm[it][:, jt * 128:(jt + 1) * 128],
                    ident_bf[:])
            at_bf = small.tile([128, C], bf16, tag=f"atbf{jt}", name=f"atb{jt}")
            nc.scalar.copy(at_bf[:], at_ps[:])
            nc.tensor.matmul(
                ntc_ps[0:87, :], lhsT=w0_sb[:, jt, :], rhs=at_bf[:],
                start=(jt == 0), stop=(jt == 1))

        lhs_m2 = small.tile([128, C], bf16, tag="lhsm2")
        rv = rv_ext
        nc.scalar.activation(
            out=lhs_m2[0:64, :], in_=ntc_ps[0:64, :], func=AF.Copy,
            scale=rv[0:64, :])
        nc.scalar.activation(
            out=lhs_m2[64:87, :], in_=ntc_ps[64:87, :], func=AF.Copy,
            scale=rv[64:87, :])
        nc.tensor.matmul(
            ntc_ps[64:66, :], lhsT=mvinv_bf[:],
            rhs=lhs_m2[0:86, :], start=True, stop=True)
        c12_sb = small.tile([128, C], bf16, tag="c12sb")
        nc.scalar.copy(c12_sb[64:66, :], ntc_ps[64:66, :])
        nc.gpsimd.dma_start(out=lhs_m2[87:89, :], in_=c12_sb[64:66, :])

        # --- M2: out = x + att_nrm @ v ---
        nadd = 0
        for it in range(2):
            for ch in range(8):
                ostg = osml.tile([128, 2048], bf16, tag="ostg",
                                 name=f"o{it}{ch}")
                for j in range(2):
                    off = ch * 2048 + j * 1024
                    o_ps = psO.tile([128, 1024], f32, tag="oP",
                                    name=f"op{it}{ch}{j}")
                    for h in range(2):
                        nc.tensor.matmul(
                            o_ps[:, h * 512:(h + 1) * 512],
                            lhsT=lhs_m2[0:89, it * 128:(it + 1) * 128],
                            rhs=rhs_m2[0:89, off + h * 512:off + (h + 1) * 512],
                            start=True, stop=True)
                    nadd += 1
                    nc.vector.tensor_tensor(
                        out=ostg[:, j * 1024:(j + 1) * 1024], in0=o_ps[:],
                        in1=x_sb[:, it, off:off + 1024], op=OP.add)
                seng = nc.sync if ch % 2 == 0 else nc.gpsimd
                seng.dma_start(
                    out=out_d[it * 128:(it + 1) * 128,
                              ch * 2048:(ch + 1) * 2048],
                    in_=ostg[:])

    nc.compile()
    return nc


def _host_prep(x, gamma, beta, w_qkv, b_qkv):
    xf = np.asarray(x, np.float32).reshape(B, C, S)
    gam = np.asarray(gamma, np.float32).reshape(-1)
    bet = np.asarray(beta, np.float32).reshape(-1)
    w_qkv = np.asarray(w_qkv, np.float32)
    b_qkv = np.asarray(b_qkv, np.float32)
    w_q, w_k, w_v = w_qkv[:C], w_qkv[C:2 * C], w_qkv[2 * C:]
    b_q, b_k, b_v = b_qkv[:C], b_qkv[C:2 * C], b_qkv[2 * C:]

    ii = np.arange(C)
    eqt = np.zeros((97, C), np.float32)
    eqt[ii // 3, ii] = w_q
    eqt[96] = b_q
    ekt = np.zeros((86, C), np.float32)
    ekt[(C + ii) // 3 - 85, ii] = w_k
    w0 = np.zeros((C, 87), np.float32)
    w0[ii, (2 * C + ii) // 3 - 170] = w_v
    w0[:, 86] = b_v
    # packed [128, 2*87]: w0p[p, jt*87+j] = w0[jt*128+p, j]
    w0 = np.ascontiguousarray(
        w0.reshape(2, 128, 87).transpose(1, 0, 2).reshape(128, 174)
    ).astype(_BF)

    sc = np.zeros((1, 8), np.float32)
    sc[0, :5] = [gam.sum(), (gam * gam).sum(), (gam * bet).sum(),
                 bet.sum(), (bet * bet).sum()]

    in_maps = []
    for r in range(NCORES):
        b, half = r // 2, r % 2
        sl = slice(half * SH, (half + 1) * SH)
        gl = gam[sl]
        bl = bet[sl]
        gb1r = np.stack([np.ones(SH, np.float32), -gl, bl], 0)

        xl = xf[b][:, sl]                       # [256, 16384]
        xtl = np.ascontiguousarray(xl.T)        # [16384, 256]
        blocks = np.empty((SH, 176), np.float32)
        blocks[:, 0:86] = xtl[:, 0:86]
        blocks[:, 86] = gl * gl
        blocks[:, 87] = gl * bl
        blocks[:, 88] = gl
        blocks[:, 89:175] = xtl[:, 85:171]
        blocks[:, 175] = 1.0
        xt = blocks.reshape(NST, 128, 176).transpose(1, 0, 2)
        xt = np.ascontiguousarray(xt.reshape(128, NST * 176)).astype(_BF)

        g2c = (gl * gl).reshape(NST, 128).T     # [128, NST]
        g2e = np.empty((128, NST, 90), np.float32)
        g2e[:, :, 0:3] = 1.0
        g2e[:, :, 3:89] = g2c[:, :, None]
        g2e[:, :, 89] = 1.0
        g2e = np.ascontiguousarray(g2e.reshape(128, NST * 90)).astype(_BF)

        in_maps.append({
            "xs": np.ascontiguousarray(xl).astype(_BF),
            "xt": xt,
            "g2e": g2e,
            "gb1r": gb1r.astype(_BF),
            "eqt": eqt,
            "ekt": ekt,
            "w0": w0,
            "bk": b_k.reshape(1, C).copy(),
            "sc": sc,
        })
    return in_maps


def kernel(x, gamma, beta, w_qkv, b_qkv):
    from concourse.bass_utils import run_bass_kernel_spmd

    if "nc" not in _cache:
        _cache["nc"] = _build_program()
    nc = _cache["nc"]

    in_maps = _host_prep(x, gamma, beta, w_qkv, b_qkv)
    res = run_bass_kernel_spmd(nc, in_maps, core_ids=list(range(NCORES)))
    out = np.empty((B, C, S), np.float32)
    for r in range(NCORES):
        b, half = r // 2, r % 2
        out[b][:, half * SH:(half + 1) * SH] = np.asarray(
            res.results[r]["out"]).astype(np.float32)
    return out.reshape(np.asarray(x).shape)


if __name__ == "__main__":
    rng = np.random.default_rng(0)
    inputs = {
        "x": rng.standard_normal((B, C, 32, 32, 32)).astype(np.float32),
        "gamma": (1 + 0.1 * rng.standard_normal((32, 32, 32))).astype(np.float32),
        "beta": (0.1 * rng.standard_normal((32, 32, 32))).astype(np.float32),
        "w_qkv": (0.5 * rng.standard_normal(3 * C)).astype(np.float32),
        "b_qkv": (0.05 * rng.standard_normal(3 * C)).astype(np.float32),
    }
    o = kernel(**inputs)
    print("out", o.shape, o.dtype, float(np.abs(o).mean()))


# revision 26
# speedup vs baseline: 1.2655x; 1.2655x over previous
"""Channel-self-attention (LayerNorm + grouped-1x1-qkv + channel softmax attn
+ residual) on 8 TRN2 NeuronCores.

Strategy (v3): pair-sharding — 2 cores per batch, each core owns one
spatial half (16384 of 32768). One ~34 KB 2-rank Mesh AllReduce per core.

Per core:
 - x half-shard [256, 16384] bf16 resident in SBUF (channel-major)
 - host also sends x TRANSPOSED (spatial-major, bf16) packed per 128-row
   stile as [x_A(86) | g2 gb g (3) | x_K(86)] so the Gram matmul needs NO
   on-chip transposes:
     lhsT = [g2 gb g | g2*x_K]   (g2*x_K built by 8 bulk chunk DVE mults
                                  against a host-replicated gamma^2 tile)
     rhs  = the raw packed stile
     out  = [89,175]: rows 0..2 x cols 0..85 = tgh_A, rows 3..88 = P^T,
            rows 0..2 x cols 89..174 = tgh_K
 - stats: Sum x via DVE reduce, Sum x^2 via Scalar Square+accum (idle
   engine), replacing bn_stats
 - ONE AllReduce (Gram + tgh + stats, 33.7 KB) within the batch pair
 - logits from the Gram expansion of the LayerNorm algebra; softmax
   normalization folded into att before the transpose, so the epilogue is
   a plain  out = x + att_nrm @ v  residual add (split DVE/GpSimd), with
   bf16 output upcast on host
"""
import sys

sys.path.insert(0, "/opt/trn_rl_repo")

import numpy as np
import ml_dtypes

B, C = 4, 256
S = 32 * 32 * 32          # 32768 global spatial
NCORES = 8
SH = S // 2               # 16384 per-core spatial half
NST = SH // 128           # 128 stiles
NCH = 8                   # Gram stream chunks
CST = NST // NCH          # 16 stiles per chunk
EPS = 1e-5
SCALE = float(S) ** -0.5

_BF = ml_dtypes.bfloat16

_cache = {}


def _build_program():
    from contextlib import ExitStack
    import concourse.bass as bass
    import concourse.bacc as bacc
    import concourse.tile as tile
    from concourse import mybir, masks

    f32 = mybir.dt.float32
    bf16 = mybir.dt.bfloat16
    AF = mybir.ActivationFunctionType
    OP = mybir.AluOpType
    AX = mybir.AxisListType

    nc = bacc.Bacc(
        "TRN2",
        target_bir_lowering=False,
        debug=False,
        enable_asserts=False,
        num_devices=NCORES,
    )

    # ---------------- DRAM I/O ----------------
    xs_d = nc.dram_tensor("xs", [C, SH], bf16, kind="ExternalInput")
    xt_d = nc.dram_tensor("xt", [128, NST * 176], bf16, kind="ExternalInput")
    g2e_d = nc.dram_tensor("g2e", [128, NST * 90], bf16, kind="ExternalInput")
    gb1r_d = nc.dram_tensor("gb1r", [3, SH], bf16, kind="ExternalInput")
    eqt_d = nc.dram_tensor("eqt", [97, C], f32, kind="ExternalInput")
    ekt_d = nc.dram_tensor("ekt", [86, C], f32, kind="ExternalInput")
    w0_d = nc.dram_tensor("w0", [128, 2 * 87], bf16, kind="ExternalInput")
    bk_d = nc.dram_tensor("bk", [1, C], f32, kind="ExternalInput")
    sc_d = nc.dram_tensor("sc", [1, 8], f32, kind="ExternalInput")
    out_d = nc.dram_tensor("out", [C, SH], bf16, kind="ExternalOutput")

    # AR payload layout (f32 words). M = [90,175] Gram PSUM; lhsT col 89
    # is ones so M row 89 = per-channel Sum x for A (cols 0:86) / K (89:175).
    #   [0 : 7740)        M[0:90, 0:86] row-major (tgh_A 0:3, P^T 3:89, SxA 89)
    #   [7740 : 7998)     M[0:3, 89:175] row-major (tgh_K)
    #   [7998 : 8084)     M[89, 89:175]  (Sum x_K)
    #   [8084 : 8170)     Sum x_V (DVE reduce over partitions 42:128 of ct1)
    #   [8170 : 8426)     Sum x^2 per channel
    PTOT = 8426

    with tile.TileContext(nc) as tc, ExitStack() as ctx:
        const = ctx.enter_context(tc.tile_pool(name="const", bufs=1))
        xpool = ctx.enter_context(tc.tile_pool(name="xpool", bufs=1))
        xtp = ctx.enter_context(tc.tile_pool(name="xtp", bufs=2))
        utp = ctx.enter_context(tc.tile_pool(name="utp", bufs=2))
        g2p = ctx.enter_context(tc.tile_pool(name="g2p", bufs=1))
        rhsp = ctx.enter_context(tc.tile_pool(name="rhsp", bufs=1))
        osml = ctx.enter_context(tc.tile_pool(name="osml", bufs=2))
        small = ctx.enter_context(tc.tile_pool(name="small", bufs=2))
        dram = ctx.enter_context(tc.tile_pool(name="dram", bufs=1, space="DRAM"))

        # ------------- constants / inputs to SBUF -------------
        ident = const.tile([128, 128], f32)
        masks.make_identity(nc, ident[:])
        ident_bf = const.tile([128, 128], bf16)
        masks.make_identity(nc, ident_bf[:])
        def dram_bcast(dst, src_d, nparts, nfree, off=0):
            nc.gpsimd.dma_start(
                out=dst,
                in_=bass.AP(tensor=src_d, offset=off,
                            ap=[[0, nparts], [1, nfree]]))

        # Gram streams: xt chunks on gpsimd queue; g2e resident via scalar
        g2e_sb = g2p.tile([128, NST, 90], bf16)
        nc.scalar.dma_start(out=g2e_sb[:], in_=g2e_d.ap())
        xt_sb, u2t_sb = [], []
        for c in range(NCH):
            t = xtp.tile([128, CST, 176], bf16, tag="xt", name=f"xt{c}")
            nc.gpsimd.dma_start(
                out=t[:],
                in_=xt_d[:, c * CST * 176:(c + 1) * CST * 176])
            xt_sb.append(t)
            u = utp.tile([128, CST, 90], bf16, tag="u2t", name=f"u2{c}")
            u2t_sb.append(u)

        # x resident bf16 [128, 2, 16384]; whole-ctile DMAs (contiguous rows
        # -- chunked column slices are descriptor-expensive)
        x_sb = xpool.tile([128, 2, SH], bf16)
        for ct in range(2):
            nc.sync.dma_start(
                out=x_sb[:, ct, :],
                in_=xs_d[ct * 128:(ct + 1) * 128, :])

        # ------------- Gram over 128 stiles (8 chunks) -------------
        bncP_in = dram.tile([PTOT], f32)
        bncP_out = dram.tile([PTOT], f32)

        with tc.tile_pool(name="s1ps", bufs=1, space="PSUM") as stg1ps:
            ptk_ps = stg1ps.tile([90, 175], f32)
            with tc.high_priority():
                for c in range(NCH):
                    nc.vector.tensor_tensor(
                        out=u2t_sb[c][:], in0=xt_sb[c][:, :, 86:176],
                        in1=g2e_sb[:, c * CST:(c + 1) * CST, :], op=OP.mult)
                    for j in range(CST):
                        st = c * CST + j
                        nc.tensor.matmul(
                            ptk_ps[:], lhsT=u2t_sb[c][:, j, :],
                            rhs=xt_sb[c][:, j, 0:175],
                            start=(st == 0), stop=(st == NST - 1))

            # ------- stats: Sum x_V (DVE), Sum x^2 (Scalar) -------
            sumsV = const.tile([128, 1], f32)
            sVp = const.tile([128, 4], f32)
            sqp = const.tile([128, 8], f32)
            sqs_sb = const.tile([128, 2], f32)
            for cc in range(4):
                nc.vector.reduce_sum(
                    sVp[:, cc:cc + 1],
                    x_sb[:, 1, cc * 4096:(cc + 1) * 4096], axis=AX.X)
            nc.vector.reduce_sum(sumsV[:], sVp[:], axis=AX.X)
            for ct in range(2):
                for cc in range(4):
                    scr = osml.tile([128, 4096], bf16, tag="sqscr", bufs=1,
                                    name=f"sq{ct}{cc}")
                    nc.scalar.activation(
                        out=scr[:], in_=x_sb[:, ct, cc * 4096:(cc + 1) * 4096],
                        func=AF.Square,
                        accum_out=sqp[:, 4 * ct + cc:4 * ct + cc + 1])
                nc.vector.reduce_sum(
                    sqs_sb[:, ct:ct + 1], sqp[:, 4 * ct:4 * ct + 4], axis=AX.X)

            ptk_sb = small.tile([90, 86], f32, tag="ptksb", bufs=1)
            nc.scalar.copy(ptk_sb[:], ptk_ps[0:90, 0:86])
            ptk3_sb = small.tile([90, 86], f32, tag="ptk3sb", bufs=1)
            nc.scalar.copy(ptk3_sb[:], ptk_ps[0:90, 89:175])

        # ------------- AllReduce within the batch pair -------------
        nc.gpsimd.dma_start(
            out=bncP_in[0:7740].rearrange("(p f) -> p f", f=86),
            in_=ptk_sb[:])
        nc.gpsimd.dma_start(
            out=bncP_in[7740:7998].rearrange("(p f) -> p f", f=86),
            in_=ptk3_sb[0:3, :])
        nc.gpsimd.dma_start(
            out=bncP_in[7998:8084].rearrange("(p f) -> p f", f=86),
            in_=ptk3_sb[89:90, :])
        nc.gpsimd.dma_start(
            out=bncP_in[8084:8170].rearrange("(p f) -> p f", f=1),
            in_=sumsV[42:128, :])
        nc.gpsimd.dma_start(
            out=bncP_in[8170:8426].rearrange("(t p) -> p t", p=128),
            in_=sqs_sb[:])
        nc.gpsimd.collective_compute(
            "AllReduce", OP.add,
            replica_groups=[[0, 1], [2, 3], [4, 5], [6, 7]],
            ins=[bncP_in[:].opt()], outs=[bncP_out[:].opt()])

        # ------------- late consts (DMA slots free after the Gram) ---------
        eqt_sb = const.tile([97, C], f32)
        nc.sync.dma_start(out=eqt_sb[:], in_=eqt_d.ap())
        ekt_sb = const.tile([86, C], f32)
        nc.sync.dma_start(out=ekt_sb[:], in_=ekt_d.ap())
        w0_sb = const.tile([128, 2, 87], bf16)
        nc.sync.dma_start(out=w0_sb[:], in_=w0_d.ap())
        bk_bc = const.tile([128, C], f32)
        dram_bcast(bk_bc[:], bk_d, 128, C)
        sc_bc = const.tile([128, 8], f32)
        dram_bcast(sc_bc[:], sc_d, 128, 8)
        gam_bc = const.tile([86, SH], bf16)
        dram_bcast(gam_bc[:], gb1r_d, 86, SH, off=SH)
        nc.vector.tensor_scalar_mul(gam_bc[:], gam_bc[:], -1.0)

        # ------------- rhs for M2 -------------
        # rows 0..85 = gamma*x_V (ch 170..255), 86..88 = [ones, -gamma, beta]
        rhs_m2 = rhsp.tile([128, SH], bf16)
        nc.gpsimd.dma_start(out=rhs_m2[0:86, :], in_=x_sb[42:128, 1, :])
        nc.vector.tensor_tensor(
            out=rhs_m2[0:86, :], in0=rhs_m2[0:86, :], in1=gam_bc[0:86, :],
            op=OP.mult)
        nc.gpsimd.dma_start(out=rhs_m2[86:89, :], in_=gb1r_d.ap())

        # ------------- DMA back -------------
        pt_back = const.tile([86, 86], f32)
        nc.sync.dma_start(
            out=pt_back[:],
            in_=bass.AP(tensor=bncP_out.tensor,
                        offset=bncP_out.offset + 3 * 86,
                        ap=[[86, 86], [1, 86]]))  # P^T rows 3..88 of block1
        tga = const.tile([86, 3], f32)
        nc.sync.dma_start(
            out=tga[:],
            in_=bass.AP(tensor=bncP_out.tensor, offset=bncP_out.offset,
                        ap=[[1, 86], [86, 3]]))
        tgk = const.tile([86, 3], f32)
        nc.sync.dma_start(
            out=tgk[:],
            in_=bass.AP(tensor=bncP_out.tensor,
                        offset=bncP_out.offset + 7740,
                        ap=[[1, 86], [86, 3]]))
        # stats cols: [p, {Sx,Sxx} x {A,K,V}]
        sAK = const.tile([86, 6], f32)
        nc.sync.dma_start(
            out=sAK[:, 0:1],
            in_=bass.AP(tensor=bncP_out.tensor,
                        offset=bncP_out.offset + 89,
                        ap=[[90, 86], [1, 1]]))       # Sx_A = block1 col 89
        nc.sync.dma_start(
            out=sAK[:, 1:3],
            in_=bass.AP(tensor=bncP_out.tensor,
                        offset=bncP_out.offset + 7998,
                        ap=[[1, 86], [86, 2]]))       # Sx_K, Sx_V
        nc.sync.dma_start(
            out=sAK[:, 3:6],
            in_=bass.AP(tensor=bncP_out.tensor,
                        offset=bncP_out.offset + 8170,
                        ap=[[1, 86], [85, 3]]))       # Sxx A/K/V

        invS = 1.0 / float(S)

        # --- per-channel LayerNorm scalars ---
        mAK = small.tile([86, 3], f32, tag="mAK")
        nc.vector.tensor_scalar(
            out=mAK[:], in0=sAK[:, 0:3], scalar1=invS, scalar2=None,
            op0=OP.mult)
        vAK = small.tile([86, 3], f32, tag="vAK")
        nc.vector.tensor_scalar(
            out=vAK[:], in0=sAK[:, 3:6], scalar1=invS, scalar2=EPS,
            op0=OP.mult, op1=OP.add)
        msq = small.tile([86, 3], f32, tag="msq")
        nc.vector.tensor_mul(msq[:], mAK[:], mAK[:])
        nc.vector.tensor_sub(vAK[:], vAK[:], msq[:])
        nc.scalar.activation(out=vAK[:], in_=vAK[:], func=AF.Sqrt)
        rAK = small.tile([86, 3], f32, tag="rAK")
        nc.vector.reciprocal(rAK[:], vAK[:])
        invrV = small.tile([86, 1], f32, tag="invrV")
        nc.vector.reciprocal(invrV[:], rAK[:, 2:3])
        mvinv_bf = small.tile([86, 2], bf16, tag="mvinv")
        nc.vector.tensor_copy(mvinv_bf[:, 0:1], mAK[:, 2:3])
        nc.vector.tensor_copy(mvinv_bf[:, 1:2], invrV[:])
        rv_ext = small.tile([128, 1], f32, tag="rvext")
        nc.vector.memset(rv_ext[64:128, :], 1.0)
        nc.vector.tensor_copy(rv_ext[0:86, :], rAK[:, 2:3])

        tA, gA, hA = tga[:, 0:1], tga[:, 1:2], tga[:, 2:3]
        tK, gK, hK = tgk[:, 0:1], tgk[:, 1:2], tgk[:, 2:3]
        mA, mK = mAK[:, 0:1], mAK[:, 1:2]
        rA, rK = rAK[:, 0:1], rAK[:, 1:2]
        scG1 = sc_bc[0:86, 0:1]
        scG2 = sc_bc[0:86, 1:2]
        scGb = sc_bc[0:86, 2:3]
        scB1 = sc_bc[0:86, 3:4]
        scBb = sc_bc[0:86, 4:5]

        ntK = small.tile([86, 1], f32, tag="ntK")
        nc.vector.tensor_scalar_mul(ntK[:], tK, -1.0)
        nmK = small.tile([86, 1], f32, tag="nmK")
        nc.vector.tensor_scalar_mul(nmK[:], mK, -1.0)
        g2mK = small.tile([86, 1], f32, tag="g2mK")
        nc.vector.tensor_scalar(
            out=g2mK[:], in0=mK, scalar1=scG2, scalar2=None, op0=OP.mult)
        t3c = small.tile([86, 1], f32, tag="t3c")
        nc.vector.tensor_scalar(
            out=t3c[:], in0=mK, scalar1=scGb, scalar2=None, op0=OP.mult)
        nc.vector.tensor_sub(t3c[:], gK, t3c[:])
        nc.vector.tensor_mul(t3c[:], rK, t3c[:])
        t2c = small.tile([86, 1], f32, tag="t2c")
        nc.vector.tensor_scalar(
            out=t2c[:], in0=mA, scalar1=scGb, scalar2=None, op0=OP.mult)
        nc.vector.tensor_sub(t2c[:], gA, t2c[:])
        nc.vector.tensor_mul(t2c[:], rA, t2c[:])
        syA = small.tile([86, 1], f32, tag="syA")
        nc.vector.tensor_scalar(
            out=syA[:], in0=mA, scalar1=scG1, scalar2=None, op0=OP.mult)
        nc.vector.tensor_sub(syA[:], hA, syA[:])
        nc.vector.tensor_mul(syA[:], rA, syA[:])
        nc.vector.tensor_scalar(
            out=syA[:], in0=syA[:], scalar1=scB1, scalar2=None, op0=OP.add)
        syK = small.tile([86, 1], f32, tag="syK")
        nc.vector.tensor_scalar(
            out=syK[:], in0=mK, scalar1=scG1, scalar2=None, op0=OP.mult)
        nc.vector.tensor_sub(syK[:], hK, syK[:])
        nc.vector.tensor_mul(syK[:], rK, syK[:])
        nc.vector.tensor_scalar(
            out=syK[:], in0=syK[:], scalar1=scB1, scalar2=None, op0=OP.add)

        with tc.tile_pool(name="psG1", bufs=1, space="PSUM") as psG1, \
             tc.tile_pool(name="psG2", bufs=1, space="PSUM") as psG2, \
             tc.tile_pool(name="psLog", bufs=2, space="PSUM") as psLog:

            # rows (mA, tA, rA, term2) -> transpose -> DRAM -> one bcast DMA
            pack = small.tile([86, 4], f32, tag="pack")
            nc.vector.tensor_copy(pack[:, 0:1], mA)
            nc.vector.tensor_copy(pack[:, 1:2], tA)
            nc.vector.tensor_copy(pack[:, 2:3], rA)
            nc.vector.tensor_copy(pack[:, 3:4], t2c[:])
            packT_ps = psG1.tile([4, 86], f32, tag="pT")
            nc.tensor.transpose(packT_ps[:], pack[:], ident[0:86, 0:86])
            packT = small.tile([4, 86], f32, tag="packT")
            nc.scalar.copy(packT[:], packT_ps[:])
            rows_d = dram.tile([4, 86], f32, tag="rowsd")
            nc.gpsimd.dma_start(out=rows_d[:], in_=packT[:])
            bc4 = small.tile([86, 4, 86], f32, tag="bc4")
            nc.gpsimd.dma_start(
                out=bc4[:],
                in_=bass.AP(tensor=rows_d.tensor, offset=rows_d.offset,
                            ap=[[0, 86], [86, 4], [1, 86]]))

            # --- syy ---
            syy = small.tile([86, 97], f32, tag="syy")
            nc.vector.memset(syy[:, 86:96], 0.0)
            nc.vector.scalar_tensor_tensor(
                out=syy[:, 0:86], in0=bc4[:, 0, :], scalar=ntK[:],
                in1=pt_back[:], op0=OP.mult, op1=OP.add)
            nc.vector.scalar_tensor_tensor(
                out=syy[:, 0:86], in0=bc4[:, 1, :], scalar=nmK[:],
                in1=syy[:, 0:86], op0=OP.mult, op1=OP.add)
            nc.vector.scalar_tensor_tensor(
                out=syy[:, 0:86], in0=bc4[:, 0, :], scalar=g2mK[:],
                in1=syy[:, 0:86], op0=OP.mult, op1=OP.add)
            nc.vector.scalar_tensor_tensor(
                out=syy[:, 0:86], in0=bc4[:, 2, :], scalar=rK,
                in1=syy[:, 0:86], op0=OP.mult, op1=OP.mult)
            nc.vector.tensor_add(syy[:, 0:86], syy[:, 0:86], bc4[:, 3, :])
            nc.vector.tensor_scalar(
                out=syy[:, 0:86], in0=syy[:, 0:86], scalar1=t3c[:],
                scalar2=scBb, op0=OP.add, op1=OP.add)
            nc.vector.tensor_copy(syy[:, 96:97], syK[:])

            # --- logits + softmax (recip folded into att) ---
            u_ps = psG2.tile([97, C], f32, tag="uP")
            nc.tensor.matmul(u_ps[:], lhsT=syy[:], rhs=ekt_sb[:],
                             start=True, stop=True)
            u_ext = small.tile([128, C], f32, tag="uext")
            nc.vector.memset(u_ext[64:128, :], 0.0)
            nc.vector.scalar_tensor_tensor(
                out=u_ext[0:86, :], in0=bk_bc[0:86, :], scalar=syA[:],
                in1=u_ps[0:86, :], op0=OP.mult, op1=OP.add)
            nc.vector.tensor_scalar_mul(
                u_ext[96:97, :], bk_bc[96:97, :], float(S))
            nc.vector.tensor_add(u_ext[96:97, :], u_ext[96:97, :],
                                 u_ps[96:97, :])

            att_nrm = []
            recip2 = small.tile([128, 2], f32, tag="recip2")
            z2 = small.tile([128, 2], f32, tag="z2")
            for it in range(2):
                log_ps = psLog.tile([128, 512], f32, tag="lg", name=f"lg{it}")
                nc.tensor.matmul(
                    log_ps[:, 0:C], lhsT=eqt_sb[:, it * 128:(it + 1) * 128],
                    rhs=u_ext[0:97, :], start=True, stop=True)
                rmax = small.tile([128, 1], f32, tag="rmax", name=f"rm{it}")
                nc.vector.reduce_max(rmax[:], log_ps[:, 0:C], axis=AX.X)
                nbias = small.tile([128, 1], f32, tag="nbias", name=f"nb{it}")
                nc.vector.tensor_scalar_mul(nbias[:], rmax[:], -SCALE)
                a_sb = small.tile([128, C], bf16, tag=f"attsb{it}",
                                  name=f"att{it}")
                nc.scalar.activation(
                    out=a_sb[:], in_=log_ps[:, 0:C], func=AF.Exp,
                    bias=nbias[:], scale=SCALE, accum_out=z2[:, it:it + 1])
                nc.vector.reciprocal(recip2[:, it:it + 1], z2[:, it:it + 1])
                a_nr = small.tile([128, C], bf16, tag=f"anrm{it}",
                                  name=f"an{it}")
                nc.scalar.activation(
                    out=a_nr[:], in_=a_sb[:], func=AF.Copy,
                    scale=recip2[:, it:it + 1])
                att_nrm.append(a_nr)

        # --- NT: lhs_m2 [89 rows, 256 q-ch] ---
        psNtc = ctx.enter_context(tc.tile_pool(name="psNtc", bufs=1,
                                               space="PSUM"))
        psAt = ctx.enter_context(tc.tile_pool(name="psAt", bufs=2,
                                              space="PSUM"))
        psO = ctx.enter_context(tc.tile_pool(name="psO", bufs=2,
                                             space="PSUM"))

        ntc_ps = psNtc.tile([128, C], f32, tag="ntc")
        for jt in range(2):
            at_ps = psAt.tile([128, C], bf16, tag="atp", name=f"atp{jt}")
            for it in range(2):
                nc.tensor.transpose(
                    at_ps[:, it * 128:(it + 1) * 128],
                    att_nrm[it][:, jt * 128:(jt + 1) * 128],
                    ident_bf[:])
            at_bf = small.tile([128, C], bf16, tag=f"atbf{jt}", name=f"atb{jt}")
            nc.scalar.copy(at_bf[:], at_ps[:])
            nc.tensor.matmul(
                ntc_ps[0:87, :], lhsT=w0_sb[:, jt, :], rhs=at_bf[:],
                start=(jt == 0), stop=(jt == 1))

        lhs_m2 = small.tile([128, C], bf16, tag="lhsm2")
        rv = rv_ext
        nc.scalar.activation(
            out=lhs_m2[0:64, :], in_=ntc_ps[0:64, :], func=AF.Copy,
            scale=rv[0:64, :])
        nc.scalar.activation(
            out=lhs_m2[64:87, :], in_=ntc_ps[64:87, :], func=AF.Copy,
            scale=rv[64:87, :])
        nc.tensor.matmul(
            ntc_ps[64:66, :], lhsT=mvinv_bf[:],
            rhs=lhs_m2[0:86, :], start=True, stop=True)
        c12_sb = small.tile([128, C], bf16, tag="c12sb")
        nc.scalar.copy(c12_sb[64:66, :], ntc_ps[64:66, :])
        nc.gpsimd.dma_start(out=lhs_m2[87:89, :], in_=c12_sb[64:66, :])

        # --- M2: out = x + att_nrm @ v ---
        nadd = 0
        for it in range(2):
            for ch in range(8):
                ostg = osml.tile([128, 2048], bf16, tag="ostg",
                                 name=f"o{it}{ch}")
                for j in range(2):
                    off = ch * 2048 + j * 1024
                    o_ps = psO.tile([128, 1024], f32, tag="oP",
                                    name=f"op{it}{ch}{j}")
                    for h in range(2):
                        nc.tensor.matmul(
                            o_ps[:, h * 512:(h + 1) * 512],
                            lhsT=lhs_m2[0:89, it * 128:(it + 1) * 128],
                            rhs=rhs_m2[0:89, off + h * 512:off + (h + 1) * 512],
                            start=True, stop=True)
                    nadd += 1
                    nc.vector.tensor_tensor(
                        out=ostg[:, j * 1024:(j + 1) * 1024], in0=o_ps[:],
                        in1=x_sb[:, it, off:off + 1024], op=OP.add)
                seng = nc.sync if ch % 2 == 0 else nc.gpsimd
                seng.dma_start(
                    out=out_d[it * 128:(it + 1) * 128,
                              ch * 2048:(ch + 1) * 2048],
                    in_=ostg[:])

    nc.compile()
    return nc


def _host_prep(x, gamma, beta, w_qkv, b_qkv):
    xf = np.asarray(x, np.float32).reshape(B, C, S)
    gam = np.asarray(gamma, np.float32).reshape(-1)
    bet = np.asarray(beta, np.float32).reshape(-1)
    w_qkv = np.asarray(w_qkv, np.float32)
    b_qkv = np.asarray(b_qkv, np.float32)
    w_q, w_k, w_v = w_qkv[:C], w_qkv[C:2 * C], w_qkv[2 * C:]
    b_q, b_k, b_v = b_qkv[:C], b_qkv[C:2 * C], b_qkv[2 * C:]

    ii = np.arange(C)
    eqt = np.zeros((97, C), np.float32)
    eqt[ii // 3, ii] = w_q
    eqt[96] = b_q
    ekt = np.zeros((86, C), np.float32)
    ekt[(C + ii) // 3 - 85, ii] = w_k
    w0 = np.zeros((C, 87), np.float32)
    w0[ii, (2 * C + ii) // 3 - 170] = w_v
    w0[:, 86] = b_v
    # packed [128, 2*87]: w0p[p, jt*87+j] = w0[jt*128+p, j]
    w0 = np.ascontiguousarray(
        w0.reshape(2, 128, 87).transpose(1, 0, 2).reshape(128, 174)
    ).astype(_BF)

    sc = np.zeros((1, 8), np.float32)
    sc[0, :5] = [gam.sum(), (gam * gam).sum(), (gam * bet).sum(),
                 bet.sum(), (bet * bet).sum()]

    in_maps = []
    for r in range(NCORES):
        b, half = r // 2, r % 2
        sl = slice(half * SH, (half + 1) * SH)
        gl = gam[sl]
        bl = bet[sl]
        gb1r = np.stack([np.ones(SH, np.float32), -gl, bl], 0)

        xl = xf[b][:, sl]                       # [256, 16384]
        xtl = np.ascontiguousarray(xl.T)        # [16384, 256]
        blocks = np.empty((SH, 176), np.float32)
        blocks[:, 0:86] = xtl[:, 0:86]
        blocks[:, 86] = gl * gl
        blocks[:, 87] = gl * bl
        blocks[:, 88] = gl
        blocks[:, 89:175] = xtl[:, 85:171]
        blocks[:, 175] = 1.0
        xt = blocks.reshape(NST, 128, 176).transpose(1, 0, 2)
        xt = np.ascontiguousarray(xt.reshape(128, NST * 176)).astype(_BF)

        g2c = (gl * gl).reshape(NST, 128).T     # [128, NST]
        g2e = np.empty((128, NST, 90), np.float32)
        g2e[:, :, 0:3] = 1.0
        g2e[:, :, 3:89] = g2c[:, :, None]
        g2e[:, :, 89] = 1.0
        g2e = np.ascontiguousarray(g2e.reshape(128, NST * 90)).astype(_BF)

        in_maps.append({
            "xs": np.ascontiguousarray(xl).astype(_BF),
            "xt": xt,
            "g2e": g2e,
            "gb1r": gb1r.astype(_BF),
            "eqt": eqt,
            "ekt": ekt,
            "w0": w0,
            "bk": b_k.reshape(1, C).copy(),
            "sc": sc,
        })
    return in_maps


def kernel(x, gamma, beta, w_qkv, b_qkv):
    from concourse.bass_utils import run_bass_kernel_spmd

    if "nc" not in _cache:
        _cache["nc"] = _build_program()
    nc = _cache["nc"]

    in_maps = _host_prep(x, gamma, beta, w_qkv, b_qkv)
    res = run_bass_kernel_spmd(nc, in_maps, core_ids=list(range(NCORES)))
    out = np.empty((B, C, S), np.float32)
    for r in range(NCORES):
        b, half = r // 2, r % 2
        out[b][:, half * SH:(half + 1) * SH] = np.asarray(
            res.results[r]["out"]).astype(np.float32)
    return out.reshape(np.asarray(x).shape)


if __name__ == "__main__":
    rng = np.random.default_rng(0)
    inputs = {
        "x": rng.standard_normal((B, C, 32, 32, 32)).astype(np.float32),
        "gamma": (1 + 0.1 * rng.standard_normal((32, 32, 32))).astype(np.float32),
        "beta": (0.1 * rng.standard_normal((32, 32, 32))).astype(np.float32),
        "w_qkv": (0.5 * rng.standard_normal(3 * C)).astype(np.float32),
        "b_qkv": (0.05 * rng.standard_normal(3 * C)).astype(np.float32),
    }
    o = kernel(**inputs)
    print("out", o.shape, o.dtype, float(np.abs(o).mean()))


# revision 27
# speedup vs baseline: 1.4278x; 1.1283x over previous
"""Channel-self-attention (LayerNorm + grouped-1x1-qkv + channel softmax attn
+ residual) on 8 TRN2 NeuronCores.

Strategy (v3): pair-sharding — 2 cores per batch, each core owns one
spatial half (16384 of 32768). One ~34 KB 2-rank Mesh AllReduce per core.

Per core:
 - x half-shard [256, 16384] bf16 resident in SBUF (channel-major)
 - host also sends x TRANSPOSED (spatial-major, bf16) packed per 128-row
   stile as [x_A(86) | g2 gb g (3) | x_K(86)] so the Gram matmul needs NO
   on-chip transposes:
     lhsT = [g2 gb g | g2*x_K]   (g2*x_K built by 8 bulk chunk DVE mults
                                  against a host-replicated gamma^2 tile)
     rhs  = the raw packed stile
     out  = [89,175]: rows 0..2 x cols 0..85 = tgh_A, rows 3..88 = P^T,
            rows 0..2 x cols 89..174 = tgh_K
 - stats: Sum x via DVE reduce, Sum x^2 via Scalar Square+accum (idle
   engine), replacing bn_stats
 - ONE AllReduce (Gram + tgh + stats, 33.7 KB) within the batch pair
 - logits from the Gram expansion of the LayerNorm algebra; softmax
   normalization folded into att before the transpose, so the epilogue is
   a plain  out = x + att_nrm @ v  residual add (split DVE/GpSimd), with
   bf16 output upcast on host
"""
import sys

sys.path.insert(0, "/opt/trn_rl_repo")

import numpy as np
import ml_dtypes

B, C = 4, 256
S = 32 * 32 * 32          # 32768 global spatial
NCORES = 8
SH = S // 2               # 16384 per-core spatial half
NST = SH // 128           # 128 stiles
NCH = 8                   # Gram stream chunks
CST = NST // NCH          # 16 stiles per chunk
EPS = 1e-5
SCALE = float(S) ** -0.5

_BF = ml_dtypes.bfloat16

_cache = {}


def _build_program():
    from contextlib import ExitStack
    import concourse.bass as bass
    import concourse.bacc as bacc
    import concourse.tile as tile
    from concourse import mybir, masks

    f32 = mybir.dt.float32
    bf16 = mybir.dt.bfloat16
    AF = mybir.ActivationFunctionType
    OP = mybir.AluOpType
    AX = mybir.AxisListType

    nc = bacc.Bacc(
        "TRN2",
        target_bir_lowering=False,
        debug=False,
        enable_asserts=False,
        num_devices=NCORES,
    )

    # ---------------- DRAM I/O ----------------
    xs_d = nc.dram_tensor("xs", [C, SH], bf16, kind="ExternalInput")
    xt_d = nc.dram_tensor("xt", [128, NST * 176], bf16, kind="ExternalInput")
    g2e_d = nc.dram_tensor("g2e", [128, NST * 90], bf16, kind="ExternalInput")
    gb1r_d = nc.dram_tensor("gb1r", [3, SH], bf16, kind="ExternalInput")
    eqt_d = nc.dram_tensor("eqt", [97, C], f32, kind="ExternalInput")
    ekt_d = nc.dram_tensor("ekt", [86, C], f32, kind="ExternalInput")
    w0_d = nc.dram_tensor("w0", [128, 2 * 87], bf16, kind="ExternalInput")
    bk_d = nc.dram_tensor("bk", [1, C], f32, kind="ExternalInput")
    sc_d = nc.dram_tensor("sc", [1, 8], f32, kind="ExternalInput")
    out_d = nc.dram_tensor("out", [C, SH], bf16, kind="ExternalOutput")

    # AR payload layout (f32 words). M = [90,175] Gram PSUM; lhsT col 89
    # is ones so M row 89 = per-channel Sum x for A (cols 0:86) / K (89:175).
    #   [0 : 7740)        M[0:90, 0:86] row-major (tgh_A 0:3, P^T 3:89, SxA 89)
    #   [7740 : 7998)     M[0:3, 89:175] row-major (tgh_K)
    #   [7998 : 8084)     M[89, 89:175]  (Sum x_K)
    #   [8084 : 8170)     Sum x_V (DVE reduce over partitions 42:128 of ct1)
    #   [8170 : 8426)     Sum x^2 per channel
    PTOT = 8426

    with tile.TileContext(nc) as tc, ExitStack() as ctx:
        const = ctx.enter_context(tc.tile_pool(name="const", bufs=1))
        xpool = ctx.enter_context(tc.tile_pool(name="xpool", bufs=1))
        xtp = ctx.enter_context(tc.tile_pool(name="xtp", bufs=2))
        utp = ctx.enter_context(tc.tile_pool(name="utp", bufs=2))
        g2p = ctx.enter_context(tc.tile_pool(name="g2p", bufs=1))
        rhsp = ctx.enter_context(tc.tile_pool(name="rhsp", bufs=1))
        osml = ctx.enter_context(tc.tile_pool(name="osml", bufs=2))
        small = ctx.enter_context(tc.tile_pool(name="small", bufs=2))
        dram = ctx.enter_context(tc.tile_pool(name="dram", bufs=1, space="DRAM"))

        # ------------- constants / inputs to SBUF -------------
        ident = const.tile([128, 128], f32)
        masks.make_identity(nc, ident[:])
        ident_bf = const.tile([128, 128], bf16)
        masks.make_identity(nc, ident_bf[:])
        eqt_sb = const.tile([97, C], f32)
        nc.sync.dma_start(out=eqt_sb[:], in_=eqt_d.ap())
        ekt_sb = const.tile([86, C], f32)
        nc.sync.dma_start(out=ekt_sb[:], in_=ekt_d.ap())
        w0_sb = const.tile([128, 2, 87], bf16)
        nc.sync.dma_start(out=w0_sb[:], in_=w0_d.ap())

        def dram_bcast(dst, src_d, nparts, nfree, off=0):
            nc.gpsimd.dma_start(
                out=dst,
                in_=bass.AP(tensor=src_d, offset=off,
                            ap=[[0, nparts], [1, nfree]]))

        bk_bc = const.tile([128, C], f32)
        dram_bcast(bk_bc[:], bk_d, 128, C)
        sc_bc = const.tile([128, 8], f32)
        dram_bcast(sc_bc[:], sc_d, 128, 8)
        gam_bc = const.tile([128, SH], bf16)
        dram_bcast(gam_bc[:], gb1r_d, 128, SH, off=SH)
        nc.vector.tensor_scalar_mul(gam_bc[:], gam_bc[:], -1.0)

        # Gram streams: xt chunks on gpsimd queue; g2e resident via scalar
        g2e_sb = g2p.tile([128, NST, 90], bf16)
        nc.scalar.dma_start(out=g2e_sb[:], in_=g2e_d.ap())
        xt_sb, u2t_sb = [], []
        for c in range(NCH):
            t = xtp.tile([128, CST, 176], bf16, tag="xt", name=f"xt{c}")
            nc.gpsimd.dma_start(
                out=t[:],
                in_=xt_d[:, c * CST * 176:(c + 1) * CST * 176])
            xt_sb.append(t)
            u = utp.tile([128, CST, 90], bf16, tag="u2t", name=f"u2{c}")
            u2t_sb.append(u)

        # x resident bf16 [128, 2, 16384]
        x_sb = xpool.tile([128, 2, SH], bf16)
        for ct in range(2):
            nc.sync.dma_start(
                out=x_sb[:, ct, :],
                in_=xs_d[ct * 128:(ct + 1) * 128, :])

        # ------------- Gram over 128 stiles (8 chunks) -------------
        bncP_in = dram.tile([PTOT], f32)
        bncP_out = dram.tile([PTOT], f32)

        with tc.tile_pool(name="s1ps", bufs=1, space="PSUM") as stg1ps:
            ptk_ps = stg1ps.tile([90, 175], f32)
            with tc.high_priority():
                for c in range(NCH):
                    nc.vector.tensor_tensor(
                        out=u2t_sb[c][:], in0=xt_sb[c][:, :, 86:176],
                        in1=g2e_sb[:, c * CST:(c + 1) * CST, :], op=OP.mult)
                    for j in range(CST):
                        st = c * CST + j
                        nc.tensor.matmul(
                            ptk_ps[:], lhsT=u2t_sb[c][:, j, :],
                            rhs=xt_sb[c][:, j, 0:175],
                            start=(st == 0), stop=(st == NST - 1))

            # ------- stats: Sum x_V (DVE), Sum x^2 (Scalar) -------
            sumsV = const.tile([128, 1], f32)
            sqp = const.tile([128, 8], f32)
            sqs_sb = const.tile([128, 2], f32)
            nc.vector.reduce_sum(
                sumsV[:], x_sb[:, 1, :], axis=AX.X)
            for ct in range(2):
                for cc in range(4):
                    scr = osml.tile([128, 4096], bf16, tag="sqscr", bufs=1,
                                    name=f"sq{ct}{cc}")
                    nc.scalar.activation(
                        out=scr[:], in_=x_sb[:, ct, cc * 4096:(cc + 1) * 4096],
                        func=AF.Square,
                        accum_out=sqp[:, 4 * ct + cc:4 * ct + cc + 1])
                nc.vector.reduce_sum(
                    sqs_sb[:, ct:ct + 1], sqp[:, 4 * ct:4 * ct + 4], axis=AX.X)

            ptk_sb = small.tile([90, 86], f32, tag="ptksb", bufs=1)
            nc.scalar.copy(ptk_sb[:], ptk_ps[0:90, 0:86])
            ptk3_sb = small.tile([90, 86], f32, tag="ptk3sb", bufs=1)
            nc.scalar.copy(ptk3_sb[:], ptk_ps[0:90, 89:175])

        # ------------- rhs for M2 (independent of AR) -------------
        # rows 0..85 = gamma*x_V (ch 170..255), 86..88 = [ones, -gamma, beta]
        rhs_m2 = rhsp.tile([128, SH], bf16)
        nc.gpsimd.dma_start(out=rhs_m2[0:86, :], in_=x_sb[42:128, 1, :])
        nc.vector.tensor_tensor(
            out=rhs_m2[0:86, :], in0=rhs_m2[0:86, :], in1=gam_bc[0:86, :],
            op=OP.mult)
        nc.gpsimd.dma_start(out=rhs_m2[86:89, :], in_=gb1r_d.ap())

        # ------------- AllReduce within the batch pair -------------
        nc.gpsimd.dma_start(
            out=bncP_in[0:7740].rearrange("(p f) -> p f", f=86),
            in_=ptk_sb[:])
        nc.gpsimd.dma_start(
            out=bncP_in[7740:7998].rearrange("(p f) -> p f", f=86),
            in_=ptk3_sb[0:3, :])
        nc.gpsimd.dma_start(
            out=bncP_in[7998:8084].rearrange("(p f) -> p f", f=86),
            in_=ptk3_sb[89:90, :])
        nc.gpsimd.dma_start(
            out=bncP_in[8084:8170].rearrange("(p f) -> p f", f=1),
            in_=sumsV[42:128, :])
        nc.gpsimd.dma_start(
            out=bncP_in[8170:8426].rearrange("(t p) -> p t", p=128),
            in_=sqs_sb[:])
        nc.gpsimd.collective_compute(
            "AllReduce", OP.add,
            replica_groups=[[0, 1], [2, 3], [4, 5], [6, 7]],
            ins=[bncP_in[:].opt()], outs=[bncP_out[:].opt()])

        # ------------- DMA back -------------
        pt_back = const.tile([86, 86], f32)
        nc.sync.dma_start(
            out=pt_back[:],
            in_=bass.AP(tensor=bncP_out.tensor,
                        offset=bncP_out.offset + 3 * 86,
                        ap=[[86, 86], [1, 86]]))  # P^T rows 3..88 of block1
        tga = const.tile([86, 3], f32)
        nc.sync.dma_start(
            out=tga[:],
            in_=bass.AP(tensor=bncP_out.tensor, offset=bncP_out.offset,
                        ap=[[1, 86], [86, 3]]))
        tgk = const.tile([86, 3], f32)
        nc.sync.dma_start(
            out=tgk[:],
            in_=bass.AP(tensor=bncP_out.tensor,
                        offset=bncP_out.offset + 7740,
                        ap=[[1, 86], [86, 3]]))
        # stats cols: [p, {Sx,Sxx} x {A,K,V}]
        sAK = const.tile([86, 6], f32)
        nc.sync.dma_start(
            out=sAK[:, 0:1],
            in_=bass.AP(tensor=bncP_out.tensor,
                        offset=bncP_out.offset + 89,
                        ap=[[90, 86], [1, 1]]))       # Sx_A = block1 col 89
        nc.sync.dma_start(
            out=sAK[:, 1:3],
            in_=bass.AP(tensor=bncP_out.tensor,
                        offset=bncP_out.offset + 7998,
                        ap=[[1, 86], [86, 2]]))       # Sx_K, Sx_V
        nc.sync.dma_start(
            out=sAK[:, 3:6],
            in_=bass.AP(tensor=bncP_out.tensor,
                        offset=bncP_out.offset + 8170,
                        ap=[[1, 86], [85, 3]]))       # Sxx A/K/V

        invS = 1.0 / float(S)

        # --- per-channel LayerNorm scalars ---
        mAK = small.tile([86, 3], f32, tag="mAK")
        nc.vector.tensor_scalar(
            out=mAK[:], in0=sAK[:, 0:3], scalar1=invS, scalar2=None,
            op0=OP.mult)
        vAK = small.tile([86, 3], f32, tag="vAK")
        nc.vector.tensor_scalar(
            out=vAK[:], in0=sAK[:, 3:6], scalar1=invS, scalar2=EPS,
            op0=OP.mult, op1=OP.add)
        msq = small.tile([86, 3], f32, tag="msq")
        nc.vector.tensor_mul(msq[:], mAK[:], mAK[:])
        nc.vector.tensor_sub(vAK[:], vAK[:], msq[:])
        nc.scalar.activation(out=vAK[:], in_=vAK[:], func=AF.Sqrt)
        rAK = small.tile([86, 3], f32, tag="rAK")
        nc.vector.reciprocal(rAK[:], vAK[:])
        invrV = small.tile([86, 1], f32, tag="invrV")
        nc.vector.reciprocal(invrV[:], rAK[:, 2:3])
        mvinv_bf = small.tile([86, 2], bf16, tag="mvinv")
        nc.vector.tensor_copy(mvinv_bf[:, 0:1], mAK[:, 2:3])
        nc.vector.tensor_copy(mvinv_bf[:, 1:2], invrV[:])
        rv_ext = small.tile([128, 1], f32, tag="rvext")
        nc.vector.memset(rv_ext[64:128, :], 1.0)
        nc.vector.tensor_copy(rv_ext[0:86, :], rAK[:, 2:3])

        tA, gA, hA = tga[:, 0:1], tga[:, 1:2], tga[:, 2:3]
        tK, gK, hK = tgk[:, 0:1], tgk[:, 1:2], tgk[:, 2:3]
        mA, mK = mAK[:, 0:1], mAK[:, 1:2]
        rA, rK = rAK[:, 0:1], rAK[:, 1:2]
        scG1 = sc_bc[0:86, 0:1]
        scG2 = sc_bc[0:86, 1:2]
        scGb = sc_bc[0:86, 2:3]
        scB1 = sc_bc[0:86, 3:4]
        scBb = sc_bc[0:86, 4:5]

        ntK = small.tile([86, 1], f32, tag="ntK")
        nc.vector.tensor_scalar_mul(ntK[:], tK, -1.0)
        nmK = small.tile([86, 1], f32, tag="nmK")
        nc.vector.tensor_scalar_mul(nmK[:], mK, -1.0)
        g2mK = small.tile([86, 1], f32, tag="g2mK")
        nc.vector.tensor_scalar(
            out=g2mK[:], in0=mK, scalar1=scG2, scalar2=None, op0=OP.mult)
        t3c = small.tile([86, 1], f32, tag="t3c")
        nc.vector.tensor_scalar(
            out=t3c[:], in0=mK, scalar1=scGb, scalar2=None, op0=OP.mult)
        nc.vector.tensor_sub(t3c[:], gK, t3c[:])
        nc.vector.tensor_mul(t3c[:], rK, t3c[:])
        t2c = small.tile([86, 1], f32, tag="t2c")
        nc.vector.tensor_scalar(
            out=t2c[:], in0=mA, scalar1=scGb, scalar2=None, op0=OP.mult)
        nc.vector.tensor_sub(t2c[:], gA, t2c[:])
        nc.vector.tensor_mul(t2c[:], rA, t2c[:])
        syA = small.tile([86, 1], f32, tag="syA")
        nc.vector.tensor_scalar(
            out=syA[:], in0=mA, scalar1=scG1, scalar2=None, op0=OP.mult)
        nc.vector.tensor_sub(syA[:], hA, syA[:])
        nc.vector.tensor_mul(syA[:], rA, syA[:])
        nc.vector.tensor_scalar(
            out=syA[:], in0=syA[:], scalar1=scB1, scalar2=None, op0=OP.add)
        syK = small.tile([86, 1], f32, tag="syK")
        nc.vector.tensor_scalar(
            out=syK[:], in0=mK, scalar1=scG1, scalar2=None, op0=OP.mult)
        nc.vector.tensor_sub(syK[:], hK, syK[:])
        nc.vector.tensor_mul(syK[:], rK, syK[:])
        nc.vector.tensor_scalar(
            out=syK[:], in0=syK[:], scalar1=scB1, scalar2=None, op0=OP.add)

        with tc.tile_pool(name="psG1", bufs=1, space="PSUM") as psG1, \
             tc.tile_pool(name="psG2", bufs=1, space="PSUM") as psG2, \
             tc.tile_pool(name="psLog", bufs=2, space="PSUM") as psLog:

            # rows (mA, tA, rA, term2) -> transpose -> DRAM -> one bcast DMA
            pack = small.tile([86, 4], f32, tag="pack")
            nc.vector.tensor_copy(pack[:, 0:1], mA)
            nc.vector.tensor_copy(pack[:, 1:2], tA)
            nc.vector.tensor_copy(pack[:, 2:3], rA)
            nc.vector.tensor_copy(pack[:, 3:4], t2c[:])
            packT_ps = psG1.tile([4, 86], f32, tag="pT")
            nc.tensor.transpose(packT_ps[:], pack[:], ident[0:86, 0:86])
            packT = small.tile([4, 86], f32, tag="packT")
            nc.scalar.copy(packT[:], packT_ps[:])
            rows_d = dram.tile([4, 86], f32, tag="rowsd")
            nc.gpsimd.dma_start(out=rows_d[:], in_=packT[:])
            bc4 = small.tile([86, 4, 86], f32, tag="bc4")
            nc.gpsimd.dma_start(
                out=bc4[:],
                in_=bass.AP(tensor=rows_d.tensor, offset=rows_d.offset,
                            ap=[[0, 86], [86, 4], [1, 86]]))

            # --- syy ---
            syy = small.tile([86, 97], f32, tag="syy")
            nc.vector.memset(syy[:, 86:96], 0.0)
            nc.vector.scalar_tensor_tensor(
                out=syy[:, 0:86], in0=bc4[:, 0, :], scalar=ntK[:],
                in1=pt_back[:], op0=OP.mult, op1=OP.add)
            nc.vector.scalar_tensor_tensor(
                out=syy[:, 0:86], in0=bc4[:, 1, :], scalar=nmK[:],
                in1=syy[:, 0:86], op0=OP.mult, op1=OP.add)
            nc.vector.scalar_tensor_tensor(
                out=syy[:, 0:86], in0=bc4[:, 0, :], scalar=g2mK[:],
                in1=syy[:, 0:86], op0=OP.mult, op1=OP.add)
            nc.vector.scalar_tensor_tensor(
                out=syy[:, 0:86], in0=bc4[:, 2, :], scalar=rK,
                in1=syy[:, 0:86], op0=OP.mult, op1=OP.mult)
            nc.vector.tensor_add(syy[:, 0:86], syy[:, 0:86], bc4[:, 3, :])
            nc.vector.tensor_scalar(
                out=syy[:, 0:86], in0=syy[:, 0:86], scalar1=t3c[:],
                scalar2=scBb, op0=OP.add, op1=OP.add)
            nc.vector.tensor_copy(syy[:, 96:97], syK[:])

            # --- logits + softmax (recip folded into att) ---
            u_ps = psG2.tile([97, C], f32, tag="uP")
            nc.tensor.matmul(u_ps[:], lhsT=syy[:], rhs=ekt_sb[:],
                             start=True, stop=True)
            u_ext = small.tile([128, C], f32, tag="uext")
            nc.vector.memset(u_ext[64:128, :], 0.0)
            nc.vector.scalar_tensor_tensor(
                out=u_ext[0:86, :], in0=bk_bc[0:86, :], scalar=syA[:],
                in1=u_ps[0:86, :], op0=OP.mult, op1=OP.add)
            nc.vector.tensor_scalar_mul(
                u_ext[96:97, :], bk_bc[96:97, :], float(S))
            nc.vector.tensor_add(u_ext[96:97, :], u_ext[96:97, :],
                                 u_ps[96:97, :])

            att_nrm = []
            recip2 = small.tile([128, 2], f32, tag="recip2")
            z2 = small.tile([128, 2], f32, tag="z2")
            for it in range(2):
                log_ps = psLog.tile([128, 512], f32, tag="lg", name=f"lg{it}")
                nc.tensor.matmul(
                    log_ps[:, 0:C], lhsT=eqt_sb[:, it * 128:(it + 1) * 128],
                    rhs=u_ext[0:97, :], start=True, stop=True)
                rmax = small.tile([128, 1], f32, tag="rmax", name=f"rm{it}")
                nc.vector.reduce_max(rmax[:], log_ps[:, 0:C], axis=AX.X)
                nbias = small.tile([128, 1], f32, tag="nbias", name=f"nb{it}")
                nc.vector.tensor_scalar_mul(nbias[:], rmax[:], -SCALE)
                a_sb = small.tile([128, C], bf16, tag=f"attsb{it}",
                                  name=f"att{it}")
                nc.scalar.activation(
                    out=a_sb[:], in_=log_ps[:, 0:C], func=AF.Exp,
                    bias=nbias[:], scale=SCALE, accum_out=z2[:, it:it + 1])
                nc.vector.reciprocal(recip2[:, it:it + 1], z2[:, it:it + 1])
                a_nr = small.tile([128, C], bf16, tag=f"anrm{it}",
                                  name=f"an{it}")
                nc.scalar.activation(
                    out=a_nr[:], in_=a_sb[:], func=AF.Copy,
                    scale=recip2[:, it:it + 1])
                att_nrm.append(a_nr)

        # --- NT: lhs_m2 [89 rows, 256 q-ch] ---
        psNtc = ctx.enter_context(tc.tile_pool(name="psNtc", bufs=1,
                                               space="PSUM"))
        psAt = ctx.enter_context(tc.tile_pool(name="psAt", bufs=2,
                                              space="PSUM"))
        psO = ctx.enter_context(tc.tile_pool(name="psO", bufs=2,
                                             space="PSUM"))

        ntc_ps = psNtc.tile([128, C], f32, tag="ntc")
        for jt in range(2):
            at_ps = psAt.tile([128, C], bf16, tag="atp", name=f"atp{jt}")
            for it in range(2):
                nc.tensor.transpose(
                    at_ps[:, it * 128:(it + 1) * 128],
                    att_nrm[it][:, jt * 128:(jt + 1) * 128],
                    ident_bf[:])
            at_bf = small.tile([128, C], bf16, tag=f"atbf{jt}", name=f"atb{jt}")
            nc.scalar.copy(at_bf[:], at_ps[:])
            nc.tensor.matmul(
                ntc_ps[0:87, :], lhsT=w0_sb[:, jt, :], rhs=at_bf[:],
                start=(jt == 0), stop=(jt == 1))

        lhs_m2 = small.tile([128, C], bf16, tag="lhsm2")
        rv = rv_ext
        nc.scalar.activation(
            out=lhs_m2[0:64, :], in_=ntc_ps[0:64, :], func=AF.Copy,
            scale=rv[0:64, :])
        nc.scalar.activation(
            out=lhs_m2[64:87, :], in_=ntc_ps[64:87, :], func=AF.Copy,
            scale=rv[64:87, :])
        nc.tensor.matmul(
            ntc_ps[64:66, :], lhsT=mvinv_bf[:],
            rhs=lhs_m2[0:86, :], start=True, stop=True)
        c12_sb = small.tile([128, C], bf16, tag="c12sb")
        nc.scalar.copy(c12_sb[64:66, :], ntc_ps[64:66, :])
        nc.gpsimd.dma_start(out=lhs_m2[87:89, :], in_=c12_sb[64:66, :])

        # --- M2: out = x + att_nrm @ v ---
        nadd = 0
        for it in range(2):
            for ch in range(8):
                ostg = osml.tile([128, 2048], bf16, tag="ostg",
                                 name=f"o{it}{ch}")
                for j in range(2):
                    off = ch * 2048 + j * 1024
                    o_ps = psO.tile([128, 1024], f32, tag="oP",
                                    name=f"op{it}{ch}{j}")
                    for h in range(2):
                        nc.tensor.matmul(
                            o_ps[:, h * 512:(h + 1) * 512],
                            lhsT=lhs_m2[0:89, it * 128:(it + 1) * 128],
                            rhs=rhs_m2[0:89, off + h * 512:off + (h + 1) * 512],
                            start=True, stop=True)
                    nadd += 1
                    nc.vector.tensor_tensor(
                        out=ostg[:, j * 1024:(j + 1) * 1024], in0=o_ps[:],
                        in1=x_sb[:, it, off:off + 1024], op=OP.add)
                nc.sync.dma_start(
                    out=out_d[it * 128:(it + 1) * 128,
                              ch * 2048:(ch + 1) * 2048],
                    in_=ostg[:])

    nc.compile()
    return nc


def _host_prep(x, gamma, beta, w_qkv, b_qkv):
    xf = np.asarray(x, np.float32).reshape(B, C, S)
    gam = np.asarray(gamma, np.float32).reshape(-1)
    bet = np.asarray(beta, np.float32).reshape(-1)
    w_qkv = np.asarray(w_qkv, np.float32)
    b_qkv = np.asarray(b_qkv, np.float32)
    w_q, w_k, w_v = w_qkv[:C], w_qkv[C:2 * C], w_qkv[2 * C:]
    b_q, b_k, b_v = b_qkv[:C], b_qkv[C:2 * C], b_qkv[2 * C:]

    ii = np.arange(C)
    eqt = np.zeros((97, C), np.float32)
    eqt[ii // 3, ii] = w_q
    eqt[96] = b_q
    ekt = np.zeros((86, C), np.float32)
    ekt[(C + ii) // 3 - 85, ii] = w_k
    w0 = np.zeros((C, 87), np.float32)
    w0[ii, (2 * C + ii) // 3 - 170] = w_v
    w0[:, 86] = b_v
    # packed [128, 2*87]: w0p[p, jt*87+j] = w0[jt*128+p, j]
    w0 = np.ascontiguousarray(
        w0.reshape(2, 128, 87).transpose(1, 0, 2).reshape(128, 174)
    ).astype(_BF)

    sc = np.zeros((1, 8), np.float32)
    sc[0, :5] = [gam.sum(), (gam * gam).sum(), (gam * bet).sum(),
                 bet.sum(), (bet * bet).sum()]

    in_maps = []
    for r in range(NCORES):
        b, half = r // 2, r % 2
        sl = slice(half * SH, (half + 1) * SH)
        gl = gam[sl]
        bl = bet[sl]
        gb1r = np.stack([np.ones(SH, np.float32), -gl, bl], 0)

        xl = xf[b][:, sl]                       # [256, 16384]
        xtl = np.ascontiguousarray(xl.T)        # [16384, 256]
        blocks = np.empty((SH, 176), np.float32)
        blocks[:, 0:86] = xtl[:, 0:86]
        blocks[:, 86] = gl * gl
        blocks[:, 87] = gl * bl
        blocks[:, 88] = gl
        blocks[:, 89:175] = xtl[:, 85:171]
        blocks[:, 175] = 1.0
        xt = blocks.reshape(NST, 128, 176).transpose(1, 0, 2)
        xt = np.ascontiguousarray(xt.reshape(128, NST * 176)).astype(_BF)

        g2c = (gl * gl).reshape(NST, 128).T     # [128, NST]
        g2e = np.empty((128, NST, 90), np.float32)
        g2e[:, :, 0:3] = 1.0
        g2e[:, :, 3:89] = g2c[:, :, None]
        g2e[:, :, 89] = 1.0
        g2e = np.ascontiguousarray(g2e.reshape(128, NST * 90)).astype(_BF)

        in_maps.append({
            "xs": np.ascontiguousarray(xl).astype(_BF),
            "xt": xt,
            "g2e": g2e,
            "gb1r": gb1r.astype(_BF),
            "eqt": eqt,
            "ekt": ekt,
            "w0": w0,
            "bk": b_k.reshape(1, C).copy(),
            "sc": sc,
        })
    return in_maps


def kernel(x, gamma, beta, w_qkv, b_qkv):
    from concourse.bass_utils import run_bass_kernel_spmd

    if "nc" not in _cache:
        _cache["nc"] = _build_program()
    nc = _cache["nc"]

    in_maps = _host_prep(x, gamma, beta, w_qkv, b_qkv)
    res = run_bass_kernel_spmd(nc, in_maps, core_ids=list(range(NCORES)))
    out = np.empty((B, C, S), np.float32)
    for r in range(NCORES):
        b, half = r // 2, r % 2
        out[b][:, half * SH:(half + 1) * SH] = np.asarray(
            res.results[r]["out"]).astype(np.float32)
    return out.reshape(np.asarray(x).shape)


if __name__ == "__main__":
    rng = np.random.default_rng(0)
    inputs = {
        "x": rng.standard_normal((B, C, 32, 32, 32)).astype(np.float32),
        "gamma": (1 + 0.1 * rng.standard_normal((32, 32, 32))).astype(np.float32),
        "beta": (0.1 * rng.standard_normal((32, 32, 32))).astype(np.float32),
        "w_qkv": (0.5 * rng.standard_normal(3 * C)).astype(np.float32),
        "b_qkv": (0.05 * rng.standard_normal(3 * C)).astype(np.float32),
    }
    o = kernel(**inputs)
    print("out", o.shape, o.dtype, float(np.abs(o).mean()))
